# revision 10
# baseline (speedup 1.0000x reference)
"""Trainium2 Bass kernel: fused attention block (QKV proj + QK-norm + RoPE +
causal SDPA + out proj), tensor-parallel over 16 heads across 8 NeuronCores.

v2 strategy (on top of the bf16 feature-major baseline):
  - All large GEMMs run as fp8e4 DoubleRow matmuls (cost-model rate 4x bf16)
    with two-plane hi/lo error compensation: x, w_qkv, w_out and ctx are each
    split into hi = e4m3(t) and lo = e4m3(t - hi); the three products
    hi*hi + lo*hi + hi*lo reconstruct bf16-accuracy at 0.75x the bf16 PE
    cost.  Weights are pre-scaled by 32 on host so their fp8 encodings stay
    in the normal range; the 32 cancels through the RMS-norm (q,k), the
    softmax normalization (v) and the final output copy (w_out).
  - Off-diagonal softmax: P = exp(s - 5) is written by the activation
    directly as fp8 (P quantization noise cancels between the numerator and
    denominator of softmax; measured harmless), and the sums / P@v matmuls
    run as DoubleRow pairs over consecutive 128-key chunks with v kept as
    hi/lo fp8 planes.  Diagonal blocks keep the bf16 path (narrowed frees +
    multiplicative causal mask).
  - Phases are interleaved per batch: project(b) -> attention(b), so the
    per-(batch,head) AllToAll chain (15us fixed cost each, serialized on
    the collective cores) overlaps projection of the next batch; the out
    projection runs last against hi/lo ctx planes pulled from the exchange,
    streaming w_out fp8 planes from HBM per output-column block.
  - q/k stay bf16 (scores at full PE rate); RMS-norm rstd rides the PE via
    ones-matmul broadcasts as in the baseline; rope tables bf16.
  - PSUM is tag-shared across the interleaved phases: "pair" (qkv-proj
    groups + score pairs), "s1" (ssum + softmax sums), "bc" (rstd + recip
    broadcasts), "cv" (ctx accumulator + v projection) -- exactly 16KB.
"""
import sys

sys.path.insert(0, "/opt/trn_rl_repo")
import numpy as np

import concourse.bacc as bacc
import concourse.mybir as mybir
from concourse.bass_utils import run_bass_kernel_spmd
from concourse.tile import TileContext

F32 = mybir.dt.float32
F32R = mybir.dt.float32r
BF16 = mybir.dt.bfloat16
FP8 = mybir.dt.float8e4
AF = mybir.ActivationFunctionType
DR = mybir.MatmulPerfMode.DoubleRow
MUL = mybir.AluOpType.mult
ADD = mybir.AluOpType.add

NCORES = 8
B, N, DM = 2, 2048, 2048
H, D = 16, 128
HLOC = H // NCORES          # 2 heads per core
T = B * N                   # 4096 flattened tokens
TCH = 8                     # token chunks of 512
KKN = DM // 128             # 16 dm chunks
KKP = KKN // 2              # 8 dm chunk-pairs
HSL = N // NCORES           # 256 tokens per core per batch after a2a
SC = 4.0                    # ctx pre-quant scale
EBIAS = -5.0                # exp bias: P = exp(s-5), cancels in softmax

SWAP16 = [(i + 16) % 32 for i in range(32)]  # rope partner swap mask

_CACHED = {}


def build():
    if "nc" in _CACHED:
        return _CACHED["nc"]
    nc = bacc.Bacc("TRN2", target_bir_lowering=False)
    for val, name in ((1e-6, "eps"), (EBIAS, "ebias")):
        t = nc.alloc_sbuf_tensor(f"const-{name}", [128, 1], F32)
        nc.gpsimd.memset(t.ap(), val)
        nc.const_aps.aps[(F32, val)] = t.ap()
    nc.all_engine_barrier()

    # x hi/lo planes, feature-major: [p, kk, t]
    xqh = nc.dram_tensor("xqh", [128, KKN, T], FP8, kind="ExternalInput")
    xql = nc.dram_tensor("xql", [128, KKN, T], FP8, kind="ExternalInput")
    # w_qkv hi/lo: [p, kkpair, parity, 4oc*128]
    wqkh = nc.dram_tensor("wqkh", [128, KKP, 2, 512], FP8, kind="ExternalInput")
    wqkl = nc.dram_tensor("wqkl", [128, KKP, 2, 512], FP8, kind="ExternalInput")
    wvh = nc.dram_tensor("wvh", [128, KKP, 2, 256], FP8, kind="ExternalInput")
    wvl = nc.dram_tensor("wvl", [128, KKP, 2, 256], FP8, kind="ExternalInput")
    # w_out hi/lo: [p, oc, core, hh, 512]
    woh = nc.dram_tensor("woh", [128, 4, NCORES, 2, 512], FP8, kind="ExternalInput")
    wol = nc.dram_tensor("wol", [128, 4, NCORES, 2, 512], FP8, kind="ExternalInput")
    csd = nc.dram_tensor("csd", [128, 4, N], BF16, kind="ExternalInput")
    masks = nc.dram_tensor("masks", [128, 4 * 512], BF16, kind="ExternalInput")
    ones_col = nc.dram_tensor("ones_col", [128, 1], F32R, kind="ExternalInput")
    ones_colb = nc.dram_tensor("ones_colb", [128, 1], BF16, kind="ExternalInput")
    ones8 = nc.dram_tensor("ones8", [128, 2, 16], FP8, kind="ExternalInput")
    ones_row = nc.dram_tensor("ones_row", [1, 128], F32R, kind="ExternalInput")
    sc_row = nc.dram_tensor("sc_row", [1, 128], F32R, kind="ExternalInput")
    # out[b, p, tt, oc, c] = y[b, core*256 + tt*128 + p, oc*512 + c]
    out = nc.dram_tensor("out", [B, 128, 2, 4, 512], F32, kind="ExternalOutput")

    with TileContext(nc) as tc, nc.allow_low_precision(reason="fp8/bf16 storage"):
        with (
            tc.tile_pool(name="acts", bufs=1) as acts,
            tc.tile_pool(name="dram", bufs=1, space="DRAM") as dpool,
            tc.tile_pool(name="qkp", bufs=1) as qkp,
            tc.tile_pool(name="cxp", bufs=1) as cxp,
        ):
            onc = acts.tile([128, 1], F32R, tag="onc")
            nc.sync.dma_start(onc[:], ones_col[:])
            oncb = acts.tile([128, 1], BF16, tag="oncb")
            nc.sync.dma_start(oncb[:], ones_colb[:])
            on8 = acts.tile([128, 2, 16], FP8, tag="on8")
            nc.sync.dma_start(on8[:], ones8[:])
            onr = acts.tile([1, 128], F32R, tag="onr")
            nc.sync.dma_start(onr[:], ones_row[:])
            onr2 = acts.tile([1, 128], F32R, tag="onr2")
            nc.sync.dma_start(onr2[:], sc_row[:])

            a2a_in = [[dpool.tile([1024, 2, HSL], FP8, tag=f"a2a_in{b}_{h}",
                                  name=f"a2a_in{b}_{h}") for h in range(HLOC)]
                      for b in range(B)]
            a2a_out = [[dpool.tile([1024, 2, HSL], FP8, tag=f"a2a_out{b}_{h}",
                                   name=f"a2a_out{b}_{h}") for h in range(HLOC)]
                       for b in range(B)]

            qf = [qkp.tile([128, TCH, 512], BF16, tag=f"qf{h}", name=f"qf{h}")
                  for h in range(HLOC)]
            kf = [qkp.tile([128, TCH, 512], BF16, tag=f"kf{h}", name=f"kf{h}")
                  for h in range(HLOC)]
            # v: bf16 copy (diagonal path) + hi/lo fp8 planes (DoubleRow)
            vbs = [qkp.tile([128, 16, 256], BF16, tag=f"vb{b}", name=f"vb{b}")
                   for b in range(B)]
            vhl = [qkp.tile([128, 16, 2, 256], FP8, tag=f"vhl{b}",
                            name=f"vhl{b}") for b in range(B)]
            masks_t = qkp.tile([128, 4 * 512], BF16, tag="masks")
            cst = qkp.tile([128, 4, N], BF16, tag="cst")
            cxb = [None, None]

            with (
                tc.tile_pool(name="wts", bufs=1) as wts,
                tc.tile_pool(name="xbp", bufs=2) as xbp,
                tc.tile_pool(name="psP", bufs=2, space="PSUM") as psP,
                tc.tile_pool(name="psS1", bufs=1, space="PSUM") as psS1,
                tc.tile_pool(name="psB", bufs=1, space="PSUM") as psB,
                tc.tile_pool(name="psC", bufs=2, space="PSUM") as psC,
                tc.tile_pool(name="nrm", bufs=1) as nrm,
                tc.tile_pool(name="Pp", bufs=2) as Pp,
                tc.tile_pool(name="P8p", bufs=3) as P8p,
                tc.tile_pool(name="ctmp", bufs=1) as ctmp,
            ):
                wqkh_t = wts.tile([128, KKP, 2, 512], FP8, tag="wqkh")
                for q4 in range(2):
                    nc.scalar.dma_start(wqkh_t[:, 4 * q4:4 * (q4 + 1)],
                                        wqkh[:, 4 * q4:4 * (q4 + 1)])
                wqkl_t = wts.tile([128, KKP, 2, 512], FP8, tag="wqkl")
                nc.scalar.dma_start(wqkl_t[:], wqkl[:])
                wvh_t = wts.tile([128, KKP, 2, 256], FP8, tag="wvh")
                nc.scalar.dma_start(wvh_t[:], wvh[:])
                wvl_t = wts.tile([128, KKP, 2, 256], FP8, tag="wvl")
                nc.scalar.dma_start(wvl_t[:], wvl[:])
                nc.scalar.dma_start(masks_t[:], masks[:])
                nc.scalar.dma_start(cst[:], csd[:])

                def emit_cx(b_, hh_):
                    if cxb[b_] is None:
                        cxb[b_] = cxp.tile([128, NCORES, 2, 2, HSL], FP8,
                                           tag=f"cx{b_}", name=f"cx{b_}")
                    for pl in range(2):
                        src_ = a2a_out[b_][hh_][:, pl].rearrange(
                            "(k p) c -> p k c", k=NCORES)
                        nc.sync.dma_start(cxb[b_][:, :, hh_, pl], src_)

                def emit_a2a(b_, hh_, cp8_):
                    for pl in range(2):
                        nc.scalar.dma_start(
                            a2a_in[b_][hh_][:, pl].rearrange(
                                "(q h p) c -> p q h c", q=4, h=2),
                            cp8_[:, pl].rearrange("p q (h c) -> p q h c", h=2))
                    nc.gpsimd.collective_compute(
                        "AllToAll", mybir.AluOpType.bypass,
                        replica_groups=[list(range(NCORES))],
                        ins=[a2a_in[b_][hh_].opt()],
                        outs=[a2a_out[b_][hh_].opt()])

                def phase_a_tch(tch):
                    """project + norm + rope one 512-token chunk."""
                    if True:
                        b = tch // 4
                        pos = (tch % 4) * 512
                        cs = cst[:, :, pos:pos + 512]
                        tsl = slice(tch * 512, (tch + 1) * 512)
                        xb = xbp.tile([128, KKN, 2, 512], FP8, tag="xb")
                        if tch == 0:  # fine-grained so the PE starts early
                            for q4 in range(4):
                                ksl = slice(4 * q4, 4 * (q4 + 1))
                                nc.sync.dma_start(xb[:, ksl, 0], xqh[:, ksl, 0:512])
                                nc.sync.dma_start(xb[:, ksl, 1], xql[:, ksl, 0:512])
                        else:
                            nc.sync.dma_start(xb[:, :, 0], xqh[:, :, tsl])
                            nc.sync.dma_start(xb[:, :, 1], xql[:, :, tsl])
                        # ---- q,k projection: hi/lo compensated DoubleRow
                        pqks = []
                        for ocp in range(2):
                            pqk = psP.tile([128, 2, 512], F32, tag="pair")
                            for j in range(KKP):
                                xh = xb[:, 2 * j:2 * j + 2, 0]
                                xl = xb[:, 2 * j:2 * j + 2, 1]
                                for i in range(2):
                                    oc = 2 * ocp + i
                                    osl = slice(oc * 128, (oc + 1) * 128)
                                    wh = wqkh_t[:, j, :, osl]
                                    wl = wqkl_t[:, j, :, osl]
                                    for t3, (st, mv) in enumerate(
                                            ((wh, xh), (wl, xh), (wh, xl))):
                                        nc.tensor.matmul(
                                            pqk[:, i], st, mv,
                                            start=(j == 0 and t3 == 0),
                                            stop=(j == KKP - 1 and t3 == 2),
                                            perf_mode=DR)
                            pqks.append(pqk)

                        def qknorm(ocp):
                            for i in range(2):
                                oc = 2 * ocp + i
                                ps = pqks[ocp][:, i]
                                sqr = nrm.tile([128, 512], F32R, tag="sq")
                                nc.scalar.activation(sqr[:], ps, AF.Square)
                                ssum = psS1.tile([1, 512], F32, tag="s1")
                                nc.tensor.matmul(ssum[:], onc[:], sqr[:],
                                                 start=True, stop=True)
                                lg = nrm.tile([1, 512], F32R, tag="lg")
                                nc.scalar.activation(
                                    lg[:], ssum[:], AF.Ln,
                                    scale=1.0 / 128.0, bias=1e-6)
                                rstd = nrm.tile([1, 512], F32R, tag="rstd")
                                nc.scalar.activation(
                                    rstd[:], lg[:], AF.Exp, scale=-0.5)
                                bcq = psB.tile([128, 512], F32, tag="bc")
                                nc.tensor.matmul(bcq[:], onr[:], rstd[:],
                                                 start=True, stop=True)
                                tt = nrm.tile([128, 512], F32, tag="tt")
                                nc.vector.stream_shuffle(tt[:], ps, SWAP16)
                                r1 = nrm.tile([128, 512], F32, tag="r1")
                                nc.vector.tensor_mul(r1[:], ps, cs[:, 2 * (oc // 2)])
                                nc.vector.tensor_mul(tt[:], tt[:], cs[:, 2 * (oc // 2) + 1])
                                nc.vector.tensor_add(r1[:], r1[:], tt[:])
                                dst = (qf[0], qf[1], kf[0], kf[1])[oc]
                                nc.vector.tensor_mul(dst[:, tch], r1[:], bcq[:])

                        qknorm(0)
                        qknorm(1)
                        # ---- v: compensated DoubleRow, token-major
                        c0 = (tch % 4) * 4
                        for tt in range(4):
                            pvt = psC.tile([128, 256], F32, tag="cv")
                            csl = slice(tt * 128, (tt + 1) * 128)
                            for j in range(KKP):
                                xh = xb[:, 2 * j:2 * j + 2, 0, csl]
                                xl = xb[:, 2 * j:2 * j + 2, 1, csl]
                                h_ = wvh_t[:, j]
                                l_ = wvl_t[:, j]
                                for t3, (st, mv) in enumerate(
                                        ((xh, h_), (xl, h_), (xh, l_))):
                                    nc.tensor.matmul(
                                        pvt[:], st, mv,
                                        start=(j == 0 and t3 == 0),
                                        stop=(j == KKP - 1 and t3 == 2),
                                        perf_mode=DR)
                            # bf16 copy (diag path) + hi/lo fp8 planes
                            nc.scalar.activation(vbs[b][:, c0 + tt], pvt[:],
                                                 AF.Copy, scale=1.0 / 32.0)
                            nc.gpsimd.tensor_copy(vhl[b][:, c0 + tt, 0],
                                                  vbs[b][:, c0 + tt])
                            nc.vector.scalar_tensor_tensor(
                                vhl[b][:, c0 + tt, 1], vhl[b][:, c0 + tt, 0],
                                -1.0, vbs[b][:, c0 + tt], MUL, ADD)

                def attention_u(b, hh, fillers=None):
                    SC_LO = (0, 128, 256, 256)
                    EX_LO = (0, 128, 256, 384)
                    vb = vbs[b]
                    vp = vhl[b]
                    fillers = fillers or {}
                    if True:
                        ctxb = ctmp.tile([128, 4, 512], BF16, tag="ctxb",
                                         name=f"ctxb{b}_{hh}")
                        cp8 = ctmp.tile([128, 2, 4, 512], FP8, tag="cp8",
                                        name=f"cp8{b}_{hh}", bufs=2)
                        pend = []

                        def flush_one():
                            kind, ent = pend.pop(0)
                            if kind == "pair":
                                sums_, ctxp_, P8, j, first = ent
                                nc.tensor.matmul(
                                    sums_[:], on8[:, :, 0:1], P8[:],
                                    start=first, stop=False, perf_mode=DR)
                                for pl in range(2):
                                    nc.tensor.matmul(
                                        ctxp_[:],
                                        vp[:, 2 * j:2 * j + 2, pl,
                                           hh * 128:(hh + 1) * 128],
                                        P8[:], start=(first and pl == 0),
                                        stop=False, perf_mode=DR)
                            else:
                                sums_, ctxp_, P_ap, kk_, lo, first, last, qs_ = ent
                                nc.tensor.matmul(
                                    sums_[:, lo:512], oncb[:], P_ap,
                                    start=first, stop=last)
                                nc.tensor.matmul(
                                    ctxp_[:, lo:512],
                                    vb[:, kk_, hh * 128:(hh + 1) * 128],
                                    P_ap, start=first, stop=last)
                                if last:
                                    rcp = ctmp.tile([1, 512], F32R, tag="rcp")
                                    nc.vector.reciprocal(rcp[:], sums_[:])
                                    bc2 = psB.tile([128, 512], F32, tag="bc")
                                    nc.tensor.matmul(bc2[:], onr2[:], rcp[:],
                                                     start=True, stop=True)
                                    bc2s = ctmp.tile([128, 512], F32, tag="bc2s")
                                    nc.vector.tensor_copy(bc2s[:], bc2[:])
                                    nc.vector.tensor_mul(
                                        ctxb[:, qs_], ctxp_[:], bc2s[:])
                                    nc.gpsimd.tensor_copy(
                                        cp8[:, 0, qs_], ctxb[:, qs_])
                                    nc.vector.scalar_tensor_tensor(
                                        cp8[:, 1, qs_], cp8[:, 0, qs_],
                                        -1.0, ctxb[:, qs_], MUL, ADD)
                                    if qs_ == 3:
                                        emit_a2a(b, hh, cp8)

                        for qs in range(4):
                            tchq = b * 4 + qs
                            sums = psS1.tile([1, 512], F32, tag="s1")
                            ctxp = psC.tile([128, 512], F32, tag="cv")
                            for j in range(2 * qs):  # off-diagonal pairs
                                sps = psP.tile([128, 2, 512], F32, tag="pair")
                                for i in range(2):
                                    kk = 2 * j + i
                                    nc.tensor.matmul(
                                        sps[:, i],
                                        kf[hh][:, b * 4 + kk // 4,
                                               (kk % 4) * 128:(kk % 4 + 1) * 128],
                                        qf[hh][:, tchq], start=True, stop=True)
                                P8 = P8p.tile([128, 2, 512], FP8, tag="P8")
                                nc.scalar.activation(P8[:], sps[:], AF.Exp,
                                                     bias=EBIAS)
                                pend.append(("pair", (sums, ctxp, P8, j, j == 0)))
                                if len(pend) > 2:
                                    flush_one()
                            for r in range(4):  # diagonal, narrowed, bf16
                                kk = 4 * qs + r
                                slo, elo = SC_LO[r], EX_LO[r]
                                sps = psP.tile([128, 2, 512], F32, tag="pair")
                                nc.tensor.matmul(
                                    sps[:, 0, slo:512],
                                    kf[hh][:, b * 4 + kk // 4,
                                           (kk % 4) * 128:(kk % 4 + 1) * 128],
                                    qf[hh][:, tchq, slo:512],
                                    start=True, stop=True)
                                P = Pp.tile([128, 2, 512], BF16, tag="P")
                                nc.scalar.activation(
                                    P[:, 0, elo:512], sps[:, 0, elo:512],
                                    AF.Exp, bias=EBIAS)
                                nc.vector.tensor_mul(
                                    P[:, 0, elo:512], P[:, 0, elo:512],
                                    masks_t[:, r * 512 + elo:(r + 1) * 512])
                                pend.append(("diag", (
                                    sums, ctxp, P[:, 0, elo:512], kk, elo,
                                    qs == 0 and r == 0, r == 3, qs)))
                                if len(pend) > 2:
                                    flush_one()
                            if qs in fillers:
                                # drain before interleaving foreign PE work so
                                # ring-buffer reuse never waits on matmuls that
                                # would be emitted behind it (deadlock)
                                while pend:
                                    flush_one()
                                fillers[qs]()
                        while pend:
                            flush_one()

                def phase_e(b):
                    # pulls first: their collective waits must not park the
                    # sync queue behind this batch's weight-ring waits
                    for hh_ in range(HLOC):
                        emit_cx(b, hh_)
                    cx = cxb[b]
                    wtags = ("qf0", "qf1", "kf0", "kf1")
                    for oc in range(4):
                        whoc = qkp.tile([128, NCORES, 2, 512], FP8,
                                        tag=wtags[(2 * oc) % 4],
                                        name=f"woh{b}_{oc}")
                        nc.sync.dma_start(whoc[:], woh[:, oc])
                        wloc = qkp.tile([128, NCORES, 2, 512], FP8,
                                        tag=wtags[(2 * oc + 1) % 4],
                                        name=f"wol{b}_{oc}")
                        nc.sync.dma_start(wloc[:], wol[:, oc])
                        for tt in range(2):
                            pso = psC.tile([128, 512], F32, tag="cv",
                                           name=f"pso{b}_{oc}_{tt}")
                            tsl = slice(tt * 128, (tt + 1) * 128)
                            for c_ in range(NCORES):
                                nc.tensor.matmul(
                                    pso[:], cx[:, c_, :, 0, tsl], whoc[:, c_],
                                    start=(c_ == 0), stop=False, perf_mode=DR)
                            for c_ in range(NCORES):
                                nc.tensor.matmul(
                                    pso[:], cx[:, c_, :, 1, tsl], whoc[:, c_],
                                    start=False, stop=False, perf_mode=DR)
                            for c_ in range(NCORES):
                                nc.tensor.matmul(
                                    pso[:], cx[:, c_, :, 0, tsl], wloc[:, c_],
                                    start=False, stop=(c_ == NCORES - 1),
                                    perf_mode=DR)
                            otb = ctmp.tile([128, 512], F32, tag="bc2s",
                                            name=f"ot{b}_{oc}_{tt}")
                            nc.scalar.activation(otb[:], pso[:], AF.Copy,
                                                 scale=1.0 / (32.0 * SC))
                            nc.scalar.dma_start(out[b][:, tt, oc], otb[:])

                # ==== interleaved schedule: A(b1) chunks fill the Act-bound
                # attention(b0) units; phase E trails the collective chain ====
                for t_ in range(4):
                    phase_a_tch(t_)
                attention_u(0, 0, {1: lambda: phase_a_tch(4),
                                   2: lambda: phase_a_tch(5),
                                   3: lambda: phase_a_tch(6)})
                attention_u(0, 1, {2: lambda: phase_a_tch(7)})
                attention_u(1, 0)
                attention_u(1, 1)
                phase_e(0)
                phase_e(1)

    nc.compile()
    _CACHED["nc"] = nc
    return nc


def _host_inputs(x, w_qkv, w_out, qn_g, kn_g):
    import ml_dtypes

    E4 = ml_dtypes.float8_e4m3
    bf16 = ml_dtypes.bfloat16
    f32 = np.float32

    x = np.asarray(x, dtype=f32)
    w_qkv = np.asarray(w_qkv, dtype=f32)
    w_out = np.asarray(w_out, dtype=f32)
    qn_g = np.asarray(qn_g, dtype=f32)
    kn_g = np.asarray(kn_g, dtype=f32)

    def split8(t):
        hi = np.ascontiguousarray(t).astype(E4)
        lo = (t - hi.astype(f32)).astype(E4)
        return hi, lo

    # head-dim permutation: 16-wide even/odd interleave so the rope partner
    # lives 16 partitions away within the same 32-partition quadrant
    perm = np.empty(D, dtype=np.int64)
    for q in range(4):
        perm[32 * q:32 * q + 16] = 2 * np.arange(16 * q, 16 * q + 16)
        perm[32 * q + 16:32 * q + 32] = 2 * np.arange(16 * q, 16 * q + 16) + 1

    # x planes: [128, KKN, T]
    xT = np.ascontiguousarray(
        x.reshape(T, DM).T.reshape(KKN, 128, T).transpose(1, 0, 2))
    xqh, xql = split8(xT)

    # w_out planes: [128 p(d within head), 4 oc, 16 head, 512] -> (core, hh)
    woT = np.ascontiguousarray(
        (32.0 * w_out).T.reshape(H, 128, 4, 512).transpose(1, 2, 0, 3))
    wh_, wl_ = split8(woT)
    woh = np.ascontiguousarray(wh_.reshape(128, 4, NCORES, 2, 512))
    wol = np.ascontiguousarray(wl_.reshape(128, 4, NCORES, 2, 512))

    # rope tables (position within a batch), permuted rows, gains folded
    inv = 1.0 / (10000.0 ** (np.arange(0, D, 2, dtype=np.float64) / D))
    ang = np.arange(N, dtype=np.float64)[:, None] * inv[None, :]
    cosn = np.empty((D, N))
    sinn = np.empty((D, N))
    c = np.cos(ang).T
    s = np.sin(ang).T
    cosn[0::2] = c
    cosn[1::2] = c
    sinn[0::2] = -s
    sinn[1::2] = s
    qscale = 1.0 / np.sqrt(np.float64(D))
    partner_nat = np.arange(D) ^ 1

    def tables(g, scale):
        g = g.astype(np.float64)
        C = (cosn * g[:, None] * scale)[perm].astype(f32)
        S = ((sinn * g[partner_nat][:, None] * scale)[perm]).astype(f32)
        return C, S

    cqt, sqt = tables(qn_g, qscale)
    ckt, skt = tables(kn_g, 1.0)
    csd = np.ascontiguousarray(
        np.stack([cqt, sqt, ckt, skt], axis=1)).astype(bf16)  # [128,4,N]

    p = np.arange(128)[:, None]
    j = np.arange(512)[None, :]
    masks = np.concatenate(
        [(128 * r + p <= j).astype(f32) for r in range(4)],
        axis=1).astype(bf16)

    shared = {
        "xqh": xqh, "xql": xql, "woh": woh, "wol": wol, "csd": csd,
        "masks": masks,
        "ones_col": np.ones((128, 1), f32),
        "ones_colb": np.ones((128, 1), bf16),
        "ones8": np.ones((128, 2, 16), f32).astype(E4),
        "ones_row": np.ones((1, 128), f32),
        "sc_row": np.full((1, 128), SC, f32),
    }
    in_maps = []
    for c_ in range(NCORES):
        hs = [HLOC * c_ + i for i in range(HLOC)]
        q_rows = np.concatenate([(0 * H + h) * D + perm for h in hs])
        k_rows = np.concatenate([(1 * H + h) * D + perm for h in hs])
        v_rows = np.concatenate([(2 * H + h) * D + np.arange(D) for h in hs])
        wqk_c = np.ascontiguousarray(
            (32.0 * w_qkv)[np.concatenate([q_rows, k_rows]), :].T
            .reshape(KKP, 2, 128, 512).transpose(2, 0, 1, 3))
        wqkh_c, wqkl_c = split8(wqk_c)
        wv_c = np.ascontiguousarray(
            (32.0 * w_qkv)[v_rows, :].T
            .reshape(KKP, 2, 128, 256).transpose(2, 0, 1, 3))
        wvh_c, wvl_c = split8(wv_c)
        in_maps.append({**shared, "wqkh": wqkh_c, "wqkl": wqkl_c,
                        "wvh": wvh_c, "wvl": wvl_c})
    return in_maps


def kernel(x, w_qkv, w_out, qn_g, kn_g):
    nc = build()
    in_maps = _host_inputs(x, w_qkv, w_out, qn_g, kn_g)
    res = run_bass_kernel_spmd(nc, in_maps, list(range(NCORES)))
    out = np.empty((B, N, DM), dtype=np.float32)
    for c in range(NCORES):
        o = res.results[c]["out"]  # [B, 128, 2, 4, 512]
        o = np.asarray(o).transpose(0, 2, 1, 3, 4).reshape(B, HSL, DM)
        out[:, c * HSL:(c + 1) * HSL, :] = o
    return out


# revision 12
# speedup vs baseline: 1.2162x; 1.2162x over previous
"""Trainium2 Bass kernel: fused attention block (QKV proj + QK-norm + RoPE +
causal SDPA + out proj), tensor-parallel over 16 heads across 8 NeuronCores.

v2 strategy (on top of the bf16 feature-major baseline):
  - All large GEMMs run as fp8e4 DoubleRow matmuls (cost-model rate 4x bf16)
    with two-plane hi/lo error compensation: x, w_qkv, w_out and ctx are each
    split into hi = e4m3(t) and lo = e4m3(t - hi); the three products
    hi*hi + lo*hi + hi*lo reconstruct bf16-accuracy at 0.75x the bf16 PE
    cost.  Weights are pre-scaled by 32 on host so their fp8 encodings stay
    in the normal range; the 32 cancels through the RMS-norm (q,k), the
    softmax normalization (v) and the final output copy (w_out).
  - Off-diagonal softmax: P = exp(s - 5) is written by the activation
    directly as fp8 (P quantization noise cancels between the numerator and
    denominator of softmax; measured harmless), and the sums / P@v matmuls
    run as DoubleRow pairs over consecutive 128-key chunks with v kept as
    hi/lo fp8 planes.  Diagonal blocks keep the bf16 path (narrowed frees +
    multiplicative causal mask).
  - Phases are interleaved per batch: project(b) -> attention(b), so the
    per-(batch,head) AllToAll chain (15us fixed cost each, serialized on
    the collective cores) overlaps projection of the next batch; the out
    projection runs last against hi/lo ctx planes pulled from the exchange,
    streaming w_out fp8 planes from HBM per output-column block.
  - q/k stay bf16 (scores at full PE rate); RMS-norm rstd rides the PE via
    ones-matmul broadcasts as in the baseline; rope tables bf16.
  - PSUM is tag-shared across the interleaved phases: "pair" (qkv-proj
    groups + score pairs), "s1" (ssum + softmax sums), "bc" (rstd + recip
    broadcasts), "cv" (ctx accumulator + v projection) -- exactly 16KB.
"""
import sys

sys.path.insert(0, "/opt/trn_rl_repo")
import numpy as np

import concourse.bacc as bacc
import concourse.mybir as mybir
from concourse.bass_utils import run_bass_kernel_spmd
from concourse.tile import TileContext

F32 = mybir.dt.float32
F32R = mybir.dt.float32r
BF16 = mybir.dt.bfloat16
FP8 = mybir.dt.float8e4
AF = mybir.ActivationFunctionType
DR = mybir.MatmulPerfMode.DoubleRow
MUL = mybir.AluOpType.mult
ADD = mybir.AluOpType.add

NCORES = 8
B, N, DM = 2, 2048, 2048
H, D = 16, 128
HLOC = H // NCORES          # 2 heads per core
T = B * N                   # 4096 flattened tokens
TCH = 8                     # token chunks of 512
KKN = DM // 128             # 16 dm chunks
KKP = KKN // 2              # 8 dm chunk-pairs
HSL = N // NCORES           # 256 tokens per core per batch after a2a
SC = 4.0                    # ctx pre-quant scale
EBIAS = -5.0                # exp bias: P = exp(s-5), cancels in softmax

SWAP16 = [(i + 16) % 32 for i in range(32)]  # rope partner swap mask

_CACHED = {}


def build():
    if "nc" in _CACHED:
        return _CACHED["nc"]
    nc = bacc.Bacc("TRN2", target_bir_lowering=False)
    for val, name in ((1e-6, "eps"), (EBIAS, "ebias")):
        t = nc.alloc_sbuf_tensor(f"const-{name}", [128, 1], F32)
        nc.gpsimd.memset(t.ap(), val)
        nc.const_aps.aps[(F32, val)] = t.ap()
    nc.all_engine_barrier()

    # x hi/lo planes, feature-major: [p, kk, t]
    xqh = nc.dram_tensor("xqh", [128, KKN, T], FP8, kind="ExternalInput")
    xql = nc.dram_tensor("xql", [128, KKN, T], FP8, kind="ExternalInput")
    # w_qkv hi/lo: [p, kkpair, parity, 4oc*128]
    wqkh = nc.dram_tensor("wqkh", [128, KKP, 2, 512], FP8, kind="ExternalInput")
    wqkl = nc.dram_tensor("wqkl", [128, KKP, 2, 512], FP8, kind="ExternalInput")
    wvh = nc.dram_tensor("wvh", [128, KKP, 2, 256], FP8, kind="ExternalInput")
    wvl = nc.dram_tensor("wvl", [128, KKP, 2, 256], FP8, kind="ExternalInput")
    # w_out hi/lo: [p, oc, core, hh, 512]
    woh = nc.dram_tensor("woh", [128, 4, NCORES, 2, 512], FP8, kind="ExternalInput")
    wol = nc.dram_tensor("wol", [128, 4, NCORES, 2, 512], FP8, kind="ExternalInput")
    csd = nc.dram_tensor("csd", [128, 4, N], BF16, kind="ExternalInput")
    masks = nc.dram_tensor("masks", [128, 4 * 512], BF16, kind="ExternalInput")
    ones_col = nc.dram_tensor("ones_col", [128, 1], F32R, kind="ExternalInput")
    ones_colb = nc.dram_tensor("ones_colb", [128, 1], BF16, kind="ExternalInput")
    ones8 = nc.dram_tensor("ones8", [128, 2, 16], FP8, kind="ExternalInput")
    ones_row = nc.dram_tensor("ones_row", [1, 128], F32R, kind="ExternalInput")
    sc_row = nc.dram_tensor("sc_row", [1, 128], F32R, kind="ExternalInput")
    # out[b, p, tt, oc, c] = y[b, core*256 + tt*128 + p, oc*512 + c]
    out = nc.dram_tensor("out", [B, 128, 2, 4, 512], F32, kind="ExternalOutput")

    with TileContext(nc) as tc, nc.allow_low_precision(reason="fp8/bf16 storage"):
        with (
            tc.tile_pool(name="acts", bufs=1) as acts,
            tc.tile_pool(name="dram", bufs=1, space="DRAM") as dpool,
            tc.tile_pool(name="qkp", bufs=1) as qkp,
            tc.tile_pool(name="cxp", bufs=1) as cxp,
        ):
            onc = acts.tile([128, 1], F32R, tag="onc")
            nc.sync.dma_start(onc[:], ones_col[:])
            oncb = acts.tile([128, 1], BF16, tag="oncb")
            nc.sync.dma_start(oncb[:], ones_colb[:])
            on8 = acts.tile([128, 2, 16], FP8, tag="on8")
            nc.sync.dma_start(on8[:], ones8[:])
            onr = acts.tile([1, 128], F32R, tag="onr")
            nc.sync.dma_start(onr[:], ones_row[:])
            onr2 = acts.tile([1, 128], F32R, tag="onr2")
            nc.sync.dma_start(onr2[:], sc_row[:])

            a2a_in = [[dpool.tile([1024, 2, HSL], FP8, tag=f"a2a_in{b}_{h}",
                                  name=f"a2a_in{b}_{h}") for h in range(HLOC)]
                      for b in range(B)]
            a2a_out = [[dpool.tile([1024, 2, HSL], FP8, tag=f"a2a_out{b}_{h}",
                                   name=f"a2a_out{b}_{h}") for h in range(HLOC)]
                       for b in range(B)]

            qf = [qkp.tile([128, TCH, 512], BF16, tag=f"qf{h}", name=f"qf{h}")
                  for h in range(HLOC)]
            kf = [qkp.tile([128, TCH, 512], BF16, tag=f"kf{h}", name=f"kf{h}")
                  for h in range(HLOC)]
            # v: bf16 copy (diagonal path) + hi/lo fp8 planes (DoubleRow)
            vbs = [qkp.tile([128, 16, 256], BF16, tag=f"vb{b}", name=f"vb{b}")
                   for b in range(B)]
            vhl = [qkp.tile([128, 16, 2, 256], FP8, tag=f"vhl{b}",
                            name=f"vhl{b}") for b in range(B)]
            masks_t = qkp.tile([128, 4 * 512], BF16, tag="masks")
            cst = qkp.tile([128, 4, N], BF16, tag="cst")
            cxb = [None, None]

            with (
                tc.tile_pool(name="wts", bufs=1) as wts,
                tc.tile_pool(name="xbp", bufs=2) as xbp,
                tc.tile_pool(name="psP", bufs=2, space="PSUM") as psP,
                tc.tile_pool(name="psS1", bufs=1, space="PSUM") as psS1,
                tc.tile_pool(name="psB", bufs=1, space="PSUM") as psB,
                tc.tile_pool(name="psC", bufs=2, space="PSUM") as psC,
                tc.tile_pool(name="nrm", bufs=1) as nrm,
                tc.tile_pool(name="Pp", bufs=2) as Pp,
                tc.tile_pool(name="P8p", bufs=3) as P8p,
                tc.tile_pool(name="ctmp", bufs=1) as ctmp,
            ):
                wqkh_t = wts.tile([128, KKP, 2, 512], FP8, tag="wqkh")
                for q4 in range(2):
                    nc.scalar.dma_start(wqkh_t[:, 4 * q4:4 * (q4 + 1)],
                                        wqkh[:, 4 * q4:4 * (q4 + 1)])
                wqkl_t = wts.tile([128, KKP, 2, 512], FP8, tag="wqkl")
                nc.scalar.dma_start(wqkl_t[:], wqkl[:])
                wvh_t = wts.tile([128, KKP, 2, 256], FP8, tag="wvh")
                nc.scalar.dma_start(wvh_t[:], wvh[:])
                wvl_t = wts.tile([128, KKP, 2, 256], FP8, tag="wvl")
                nc.scalar.dma_start(wvl_t[:], wvl[:])
                nc.scalar.dma_start(masks_t[:], masks[:])
                nc.scalar.dma_start(cst[:], csd[:])

                def emit_cx(b_, hh_):
                    if cxb[b_] is None:
                        cxb[b_] = cxp.tile([128, NCORES, 2, 2, HSL], FP8,
                                           tag=f"cx{b_}", name=f"cx{b_}")
                    for pl in range(2):
                        src_ = a2a_out[b_][hh_][:, pl].rearrange(
                            "(k p) c -> p k c", k=NCORES)
                        nc.sync.dma_start(cxb[b_][:, :, hh_, pl], src_)

                def emit_a2a(b_, hh_, cp8_):
                    for pl in range(2):
                        nc.scalar.dma_start(
                            a2a_in[b_][hh_][:, pl].rearrange(
                                "(q h p) c -> p q h c", q=4, h=2),
                            cp8_[:, pl].rearrange("p q (h c) -> p q h c", h=2))
                    nc.gpsimd.collective_compute(
                        "AllToAll", mybir.AluOpType.bypass,
                        replica_groups=[list(range(NCORES))],
                        ins=[a2a_in[b_][hh_].opt()],
                        outs=[a2a_out[b_][hh_].opt()])

                def phase_a_tch(tch):
                    """project + norm + rope one 512-token chunk."""
                    if True:
                        b = tch // 4
                        pos = (tch % 4) * 512
                        cs = cst[:, :, pos:pos + 512]
                        tsl = slice(tch * 512, (tch + 1) * 512)
                        xb = xbp.tile([128, KKN, 2, 512], FP8, tag="xb")
                        if tch == 0:  # fine-grained so the PE starts early
                            for q4 in range(4):
                                ksl = slice(4 * q4, 4 * (q4 + 1))
                                nc.sync.dma_start(xb[:, ksl, 0], xqh[:, ksl, 0:512])
                                nc.sync.dma_start(xb[:, ksl, 1], xql[:, ksl, 0:512])
                        else:
                            nc.sync.dma_start(xb[:, :, 0], xqh[:, :, tsl])
                            nc.sync.dma_start(xb[:, :, 1], xql[:, :, tsl])
                        # ---- q,k projection: hi/lo compensated DoubleRow
                        pqks = []
                        for ocp in range(2):
                            pqk = psP.tile([128, 2, 512], F32, tag="pair")
                            for j in range(KKP):
                                xh = xb[:, 2 * j:2 * j + 2, 0]
                                xl = xb[:, 2 * j:2 * j + 2, 1]
                                for i in range(2):
                                    oc = 2 * ocp + i
                                    osl = slice(oc * 128, (oc + 1) * 128)
                                    wh = wqkh_t[:, j, :, osl]
                                    wl = wqkl_t[:, j, :, osl]
                                    for t3, (st, mv) in enumerate(
                                            ((wh, xh), (wl, xh), (wh, xl))):
                                        nc.tensor.matmul(
                                            pqk[:, i], st, mv,
                                            start=(j == 0 and t3 == 0),
                                            stop=(j == KKP - 1 and t3 == 2),
                                            perf_mode=DR)
                            pqks.append(pqk)

                        def qknorm(ocp):
                            for i in range(2):
                                oc = 2 * ocp + i
                                ps = pqks[ocp][:, i]
                                sqr = nrm.tile([128, 512], F32R, tag="sq")
                                nc.scalar.activation(sqr[:], ps, AF.Square)
                                ssum = psS1.tile([1, 512], F32, tag="s1")
                                nc.tensor.matmul(ssum[:], onc[:], sqr[:],
                                                 start=True, stop=True)
                                rstd = nrm.tile([1, 512], F32R, tag="rstd")
                                nc.scalar.activation(
                                    rstd[:], ssum[:], AF.Abs_reciprocal_sqrt,
                                    scale=1.0 / 128.0, bias=1e-6)
                                bcq = psB.tile([128, 512], F32, tag="bc")
                                nc.tensor.matmul(bcq[:], onr[:], rstd[:],
                                                 start=True, stop=True)
                                tt = nrm.tile([128, 512], F32, tag="tt")
                                nc.vector.stream_shuffle(tt[:], ps, SWAP16)
                                r1 = nrm.tile([128, 512], F32, tag="r1")
                                nc.vector.tensor_mul(r1[:], ps, cs[:, 2 * (oc // 2)])
                                nc.vector.tensor_mul(tt[:], tt[:], cs[:, 2 * (oc // 2) + 1])
                                nc.vector.tensor_add(r1[:], r1[:], tt[:])
                                dst = (qf[0], qf[1], kf[0], kf[1])[oc]
                                nc.vector.tensor_mul(dst[:, tch], r1[:], bcq[:])

                        qknorm(0)
                        qknorm(1)
                        # ---- v: compensated DoubleRow, token-major
                        c0 = (tch % 4) * 4
                        for tt in range(4):
                            pvt = psC.tile([128, 256], F32, tag="cv")
                            csl = slice(tt * 128, (tt + 1) * 128)
                            for j in range(KKP):
                                xh = xb[:, 2 * j:2 * j + 2, 0, csl]
                                xl = xb[:, 2 * j:2 * j + 2, 1, csl]
                                h_ = wvh_t[:, j]
                                l_ = wvl_t[:, j]
                                for t3, (st, mv) in enumerate(
                                        ((xh, h_), (xl, h_), (xh, l_))):
                                    nc.tensor.matmul(
                                        pvt[:], st, mv,
                                        start=(j == 0 and t3 == 0),
                                        stop=(j == KKP - 1 and t3 == 2),
                                        perf_mode=DR)
                            # bf16 copy (diag path) + hi/lo fp8 planes
                            nc.scalar.activation(vbs[b][:, c0 + tt], pvt[:],
                                                 AF.Copy, scale=1.0 / 32.0)
                            nc.gpsimd.tensor_copy(vhl[b][:, c0 + tt, 0],
                                                  vbs[b][:, c0 + tt])
                            nc.vector.scalar_tensor_tensor(
                                vhl[b][:, c0 + tt, 1], vhl[b][:, c0 + tt, 0],
                                -1.0, vbs[b][:, c0 + tt], MUL, ADD)

                def attention_u(b, hh, fillers=None):
                    SC_LO = (0, 128, 256, 256)
                    EX_LO = (0, 128, 256, 384)
                    vb = vbs[b]
                    vp = vhl[b]
                    fillers = fillers or {}
                    if True:
                        ctxb = ctmp.tile([128, 4, 512], BF16, tag="ctxb",
                                         name=f"ctxb{b}_{hh}")
                        cp8 = ctmp.tile([128, 2, 4, 512], FP8, tag="cp8",
                                        name=f"cp8{b}_{hh}", bufs=2)
                        pend = []

                        def flush_one():
                            kind, ent = pend.pop(0)
                            if kind == "pair":
                                sums_, ctxp_, P8, j, first = ent
                                nc.tensor.matmul(
                                    sums_[:], on8[:, :, 0:1], P8[:],
                                    start=first, stop=False, perf_mode=DR)
                                for pl in range(2):
                                    nc.tensor.matmul(
                                        ctxp_[:],
                                        vp[:, 2 * j:2 * j + 2, pl,
                                           hh * 128:(hh + 1) * 128],
                                        P8[:], start=(first and pl == 0),
                                        stop=False, perf_mode=DR)
                            else:
                                sums_, ctxp_, P_ap, kk_, lo, first, last, qs_ = ent
                                nc.tensor.matmul(
                                    sums_[:, lo:512], oncb[:], P_ap,
                                    start=first, stop=last)
                                nc.tensor.matmul(
                                    ctxp_[:, lo:512],
                                    vb[:, kk_, hh * 128:(hh + 1) * 128],
                                    P_ap, start=first, stop=last)
                                if last:
                                    rcp = ctmp.tile([1, 512], F32R, tag="rcp")
                                    nc.vector.reciprocal(rcp[:], sums_[:])
                                    bc2 = psB.tile([128, 512], F32, tag="bc")
                                    nc.tensor.matmul(bc2[:], onr2[:], rcp[:],
                                                     start=True, stop=True)
                                    bc2s = ctmp.tile([128, 512], F32, tag="bc2s")
                                    nc.vector.tensor_copy(bc2s[:], bc2[:])
                                    nc.vector.tensor_mul(
                                        ctxb[:, qs_], ctxp_[:], bc2s[:])
                                    nc.gpsimd.tensor_copy(
                                        cp8[:, 0, qs_], ctxb[:, qs_])
                                    nc.vector.scalar_tensor_tensor(
                                        cp8[:, 1, qs_], cp8[:, 0, qs_],
                                        -1.0, ctxb[:, qs_], MUL, ADD)
                                    if qs_ == 3:
                                        emit_a2a(b, hh, cp8)

                        for qs in range(4):
                            tchq = b * 4 + qs
                            sums = psS1.tile([1, 512], F32, tag="s1")
                            ctxp = psC.tile([128, 512], F32, tag="cv")
                            for j in range(2 * qs):  # off-diagonal pairs
                                sps = psP.tile([128, 2, 512], F32, tag="pair")
                                for i in range(2):
                                    kk = 2 * j + i
                                    nc.tensor.matmul(
                                        sps[:, i],
                                        kf[hh][:, b * 4 + kk // 4,
                                               (kk % 4) * 128:(kk % 4 + 1) * 128],
                                        qf[hh][:, tchq], start=True, stop=True)
                                P8 = P8p.tile([128, 2, 512], FP8, tag="P8")
                                nc.scalar.activation(P8[:], sps[:], AF.Exp,
                                                     bias=EBIAS)
                                pend.append(("pair", (sums, ctxp, P8, j, j == 0)))
                                if len(pend) > 2:
                                    flush_one()
                            for r in range(4):  # diagonal, narrowed, bf16
                                kk = 4 * qs + r
                                slo, elo = SC_LO[r], EX_LO[r]
                                sps = psP.tile([128, 2, 512], F32, tag="pair")
                                nc.tensor.matmul(
                                    sps[:, 0, slo:512],
                                    kf[hh][:, b * 4 + kk // 4,
                                           (kk % 4) * 128:(kk % 4 + 1) * 128],
                                    qf[hh][:, tchq, slo:512],
                                    start=True, stop=True)
                                P = Pp.tile([128, 2, 512], BF16, tag="P")
                                nc.scalar.activation(
                                    P[:, 0, elo:512], sps[:, 0, elo:512],
                                    AF.Exp, bias=EBIAS)
                                nc.vector.tensor_mul(
                                    P[:, 0, elo:512], P[:, 0, elo:512],
                                    masks_t[:, r * 512 + elo:(r + 1) * 512])
                                pend.append(("diag", (
                                    sums, ctxp, P[:, 0, elo:512], kk, elo,
                                    qs == 0 and r == 0, r == 3, qs)))
                                if len(pend) > 2:
                                    flush_one()
                            if qs in fillers:
                                # drain before interleaving foreign PE work so
                                # ring-buffer reuse never waits on matmuls that
                                # would be emitted behind it (deadlock)
                                while pend:
                                    flush_one()
                                fillers[qs]()
                        while pend:
                            flush_one()

                wo_t = [[None, None] for _ in range(4)]

                def load_wo1():
                    # w_out fp8 planes (8MB total) loaded ONCE into SBUF slots
                    # recycled from tiles that drain during attention(1):
                    # oc0/oc1 combined [H|L] tiles in the xb slots, oc2/oc3
                    # hi planes in the qf0/kf0 slots (free after unit (1,h0)).
                    for oc in range(2):
                        w2 = xbp.tile([128, 2, NCORES, 2, 512], FP8, tag="xb",
                                      name=f"wo2_{oc}")
                        nc.sync.dma_start(w2[:, 0], woh[:, oc])
                        nc.sync.dma_start(w2[:, 1], wol[:, oc])
                        wo_t[oc] = [w2[:, 0], w2[:, 1]]
                    for oc, tgh in ((2, "qf0"), (3, "kf0")):
                        wh_ = qkp.tile([128, NCORES, 2, 512], FP8, tag=tgh,
                                       name=f"woh_{oc}")
                        nc.sync.dma_start(wh_[:], woh[:, oc])
                        wo_t[oc][0] = wh_[:]

                def load_wo2():
                    # lo planes for oc2/oc3 into qf1/kf1, which unit (1,h1)
                    # reads until its last score matmul
                    for oc, tgl in ((2, "qf1"), (3, "kf1")):
                        wl_ = qkp.tile([128, NCORES, 2, 512], FP8, tag=tgl,
                                       name=f"wol_{oc}")
                        nc.sync.dma_start(wl_[:], wol[:, oc])
                        wo_t[oc][1] = wl_[:]

                def phase_e(b):
                    for hh_ in range(HLOC):
                        emit_cx(b, hh_)
                    cx = cxb[b]
                    for oc in range(4):
                        whoc, wloc = wo_t[oc]
                        for tt in range(2):
                            pso = psC.tile([128, 512], F32, tag="cv",
                                           name=f"pso{b}_{oc}_{tt}")
                            tsl = slice(tt * 128, (tt + 1) * 128)
                            for c_ in range(NCORES):
                                nc.tensor.matmul(
                                    pso[:], cx[:, c_, :, 0, tsl], whoc[:, c_],
                                    start=(c_ == 0), stop=False, perf_mode=DR)
                            for c_ in range(NCORES):
                                nc.tensor.matmul(
                                    pso[:], cx[:, c_, :, 1, tsl], whoc[:, c_],
                                    start=False, stop=False, perf_mode=DR)
                            for c_ in range(NCORES):
                                nc.tensor.matmul(
                                    pso[:], cx[:, c_, :, 0, tsl], wloc[:, c_],
                                    start=False, stop=(c_ == NCORES - 1),
                                    perf_mode=DR)
                            otb = ctmp.tile([128, 512], F32, tag="bc2s",
                                            name=f"ot{b}_{oc}_{tt}")
                            nc.scalar.activation(otb[:], pso[:], AF.Copy,
                                                 scale=1.0 / (32.0 * SC))
                            nc.scalar.dma_start(out[b][:, tt, oc], otb[:])

                # ==== interleaved schedule: A(b1) chunks fill the Act-bound
                # attention(b0) units; phase E trails the collective chain ====
                for t_ in range(4):
                    phase_a_tch(t_)
                attention_u(0, 0, {1: lambda: phase_a_tch(4),
                                   2: lambda: phase_a_tch(5),
                                   3: lambda: phase_a_tch(6)})
                attention_u(0, 1, {2: lambda: phase_a_tch(7)})
                attention_u(1, 0)
                load_wo1()
                attention_u(1, 1)
                load_wo2()
                phase_e(0)
                phase_e(1)

    nc.compile()
    _CACHED["nc"] = nc
    return nc


def _host_inputs(x, w_qkv, w_out, qn_g, kn_g):
    import ml_dtypes

    E4 = ml_dtypes.float8_e4m3
    bf16 = ml_dtypes.bfloat16
    f32 = np.float32

    x = np.asarray(x, dtype=f32)
    w_qkv = np.asarray(w_qkv, dtype=f32)
    w_out = np.asarray(w_out, dtype=f32)
    qn_g = np.asarray(qn_g, dtype=f32)
    kn_g = np.asarray(kn_g, dtype=f32)

    def split8(t):
        hi = np.ascontiguousarray(t).astype(E4)
        lo = (t - hi.astype(f32)).astype(E4)
        return hi, lo

    # head-dim permutation: 16-wide even/odd interleave so the rope partner
    # lives 16 partitions away within the same 32-partition quadrant
    perm = np.empty(D, dtype=np.int64)
    for q in range(4):
        perm[32 * q:32 * q + 16] = 2 * np.arange(16 * q, 16 * q + 16)
        perm[32 * q + 16:32 * q + 32] = 2 * np.arange(16 * q, 16 * q + 16) + 1

    # x planes: [128, KKN, T]
    xT = np.ascontiguousarray(
        x.reshape(T, DM).T.reshape(KKN, 128, T).transpose(1, 0, 2))
    xqh, xql = split8(xT)

    # w_out planes: [128 p(d within head), 4 oc, 16 head, 512] -> (core, hh)
    woT = np.ascontiguousarray(
        (32.0 * w_out).T.reshape(H, 128, 4, 512).transpose(1, 2, 0, 3))
    wh_, wl_ = split8(woT)
    woh = np.ascontiguousarray(wh_.reshape(128, 4, NCORES, 2, 512))
    wol = np.ascontiguousarray(wl_.reshape(128, 4, NCORES, 2, 512))

    # rope tables (position within a batch), permuted rows, gains folded
    inv = 1.0 / (10000.0 ** (np.arange(0, D, 2, dtype=np.float64) / D))
    ang = np.arange(N, dtype=np.float64)[:, None] * inv[None, :]
    cosn = np.empty((D, N))
    sinn = np.empty((D, N))
    c = np.cos(ang).T
    s = np.sin(ang).T
    cosn[0::2] = c
    cosn[1::2] = c
    sinn[0::2] = -s
    sinn[1::2] = s
    qscale = 1.0 / np.sqrt(np.float64(D))
    partner_nat = np.arange(D) ^ 1

    def tables(g, scale):
        g = g.astype(np.float64)
        C = (cosn * g[:, None] * scale)[perm].astype(f32)
        S = ((sinn * g[partner_nat][:, None] * scale)[perm]).astype(f32)
        return C, S

    cqt, sqt = tables(qn_g, qscale)
    ckt, skt = tables(kn_g, 1.0)
    csd = np.ascontiguousarray(
        np.stack([cqt, sqt, ckt, skt], axis=1)).astype(bf16)  # [128,4,N]

    p = np.arange(128)[:, None]
    j = np.arange(512)[None, :]
    masks = np.concatenate(
        [(128 * r + p <= j).astype(f32) for r in range(4)],
        axis=1).astype(bf16)

    shared = {
        "xqh": xqh, "xql": xql, "woh": woh, "wol": wol, "csd": csd,
        "masks": masks,
        "ones_col": np.ones((128, 1), f32),
        "ones_colb": np.ones((128, 1), bf16),
        "ones8": np.ones((128, 2, 16), f32).astype(E4),
        "ones_row": np.ones((1, 128), f32),
        "sc_row": np.full((1, 128), SC, f32),
    }
    in_maps = []
    for c_ in range(NCORES):
        hs = [HLOC * c_ + i for i in range(HLOC)]
        q_rows = np.concatenate([(0 * H + h) * D + perm for h in hs])
        k_rows = np.concatenate([(1 * H + h) * D + perm for h in hs])
        v_rows = np.concatenate([(2 * H + h) * D + np.arange(D) for h in hs])
        wqk_c = np.ascontiguousarray(
            (32.0 * w_qkv)[np.concatenate([q_rows, k_rows]), :].T
            .reshape(KKP, 2, 128, 512).transpose(2, 0, 1, 3))
        wqkh_c, wqkl_c = split8(wqk_c)
        wv_c = np.ascontiguousarray(
            (32.0 * w_qkv)[v_rows, :].T
            .reshape(KKP, 2, 128, 256).transpose(2, 0, 1, 3))
        wvh_c, wvl_c = split8(wv_c)
        in_maps.append({**shared, "wqkh": wqkh_c, "wqkl": wqkl_c,
                        "wvh": wvh_c, "wvl": wvl_c})
    return in_maps


def kernel(x, w_qkv, w_out, qn_g, kn_g):
    nc = build()
    in_maps = _host_inputs(x, w_qkv, w_out, qn_g, kn_g)
    res = run_bass_kernel_spmd(nc, in_maps, list(range(NCORES)))
    out = np.empty((B, N, DM), dtype=np.float32)
    for c in range(NCORES):
        o = res.results[c]["out"]  # [B, 128, 2, 4, 512]
        o = np.asarray(o).transpose(0, 2, 1, 3, 4).reshape(B, HSL, DM)
        out[:, c * HSL:(c + 1) * HSL, :] = o
    return out
